# revision 6
# baseline (speedup 1.0000x reference)
"""Trainium2 Bass kernel for AttentionAugmentation2D.

Problem: 2D self-attention with relative position logits.
  inputs [2, 48, 48, 192] -> q,k,v (64 ch each), 8 heads x 8 dims.
  Per (batch, head): logits[n, m] = q.k + w_rel + h_rel over HW=2304 positions,
  softmax over m, output = weights @ v.

Sharding: 16 independent (batch, head) units -> head h goes to core h,
each core handles both batches for its head (2 units/core).

Math trick: for key m and query n (n = i*48+j, m = i'*48+j'):
  logitsT[m, n] = sum_d k[m,d] q[n,d]
               + sum_d key_rel_w[47 + j'(m) - j(n), d] q[n,d]
               + sum_d key_rel_h[47 + i'(m) - i(n), d] q[n,d]
One matmul with contraction K=120:
  lhsT rows: [EY (48) | 0 (16) | EX (48) | kT (8)]  EY[y,m]=[m%48==y], EX[x,m]=[m//48==x]
  rhs  rows: [RW (48) | 0 (16) | RH (48) | qT (8)]  RW[y,n] = sum_d Wrel[47+y-j(n),d] q[n,d]
(48-row blocks at 32-aligned partition bases for the DVE copies that fill
them; kT/qT rows are DMA-written so any partition base is fine there.)
RW/RH are built on-chip by 48 small matmuls each (per j-class / i-class of n),
stacked into one [112, 1536] psum tile (30 groups at partitions 0.., 18 at 64..).

The logits stream (18 key-blocks x 2304 queries) is produced in uniform
[128, 1536] psum units that roll across block boundaries; each unit gets one
exp ACT op into a rolling [128, 4608] fp16 buffer (block j's exps always land
contiguously at (2304*j) % 4608).  Softmax skips max-subtraction (|logits| <~
12; exp biased by -4 which cancels in normalization).  PV matmuls accumulate
[attn_unnorm | rowsum] per 128-query block via a ones-augmented v (fp16).
"""

import sys

if "/opt/trn_rl_repo" not in sys.path:
    sys.path.insert(0, "/opt/trn_rl_repo")

import numpy as np

B = 2
HH = 48
WW = 48
NH = 8
DKH = 8
HW = HH * WW          # 2304
NB = HW // 128        # 18 query/key blocks
KDIM = 120            # contraction rows: RW 0:48, zero 48:64, RH 64:112, qT 112:120
VC = 9                # v cols + ones
UNIT = 1536           # psum unit width (3 banks)
NU = NB * HW // UNIT  # 27 units per (batch, head) pair
EBUF = 2 * HW         # rolling exp buffer cols
EXP_BIAS = -4.0

_CACHE = {}


def _build_program():
    import concourse.bass as bass
    from concourse import bacc, mybir, tile

    f16 = mybir.dt.float16
    f32 = mybir.dt.float32

    nc = bacc.Bacc(
        "TRN2",
        target_bir_lowering=False,
        debug=False,
        enable_asserts=False,
        num_devices=8,
    )

    qt_d = nc.dram_tensor("qt", [B, 8, HW], f16, kind="ExternalInput")
    kt_d = nc.dram_tensor("kt", [B, 8, HW], f16, kind="ExternalInput")
    vp_d = nc.dram_tensor("vp", [B, 128, NB * VC], f16, kind="ExternalInput")
    eyz_d = nc.dram_tensor("eyz", [64, HW], f16, kind="ExternalInput")
    ex_d = nc.dram_tensor("ex", [48, HW], f16, kind="ExternalInput")
    zz_d = nc.dram_tensor("zz", [16, HW], f16, kind="ExternalInput")
    wrelt_d = nc.dram_tensor("wrelt", [8, 95], f16, kind="ExternalInput")
    hrelt_d = nc.dram_tensor("hrelt", [8, 95], f16, kind="ExternalInput")
    out_d = nc.dram_tensor("out", [B, HW, 8], f32, kind="ExternalOutput")

    with tile.TileContext(nc) as tc:
        with (
            tc.tile_pool(name="per", bufs=1) as per_pool,
            tc.tile_pool(name="fin", bufs=2) as fin_pool,
            tc.tile_pool(name="lt", bufs=2, space="PSUM") as lt_pool,
            tc.tile_pool(name="pv", bufs=2, space="PSUM") as pv_pool,
        ):
            wrelt = per_pool.tile([8, 95], f16, tag="wrelt")
            hrelt = per_pool.tile([8, 95], f16, tag="hrelt")
            ebias = per_pool.tile([128, 1], f32, tag="ebias")
            ebuf = per_pool.tile([128, EBUF], f16, tag="ebuf")
            lhs = [per_pool.tile([KDIM, HW], f16, tag=f"lhs{b}") for b in range(B)]
            rhs = [per_pool.tile([KDIM, HW], f16, tag=f"rhs{b}") for b in range(B)]
            qts = [per_pool.tile([8, HW], f16, tag=f"qts{b}") for b in range(B)]
            vps = [per_pool.tile([128, NB * VC], f16, tag=f"vps{b}") for b in range(B)]

            nc.vector.memset(ebias[:], EXP_BIAS)

            # All input DMAs up front, spread over four DGE queues.
            nc.sync.dma_start(wrelt[:], wrelt_d.ap())
            nc.sync.dma_start(hrelt[:], hrelt_d.ap())
            for b in range(B):
                nc.sync.dma_start(qts[b][:], qt_d.ap()[b])
            nc.vector.dma_start(lhs[0][0:64, :], eyz_d.ap())
            nc.vector.dma_start(lhs[0][64:112, :], ex_d.ap())
            nc.vector.dma_start(lhs[0][112:120, :], kt_d.ap()[0])
            nc.scalar.dma_start(lhs[1][0:64, :], eyz_d.ap())
            nc.scalar.dma_start(lhs[1][64:112, :], ex_d.ap())
            nc.scalar.dma_start(lhs[1][112:120, :], kt_d.ap()[1])
            for b in range(B):
                nc.gpsimd.dma_start(rhs[b][48:64, :], zz_d.ap())
                nc.gpsimd.dma_start(rhs[b][112:120, :], qt_d.ap()[b])
                nc.gpsimd.dma_start(vps[b][:], vp_d.ap()[b])

            # ---- RW/RH assembly for both pairs (overlaps main loop startup).
            # 48 groups per table stacked in one [112,1536] psum tile:
            # group g<30 -> partitions 0:48, bank g//10, slot g%10;
            # g>=30 -> partitions 64:112, bank (g-30)//10, slot (g-30)%10.
            def asm(b, relt, dst_rows, reorder):
                pw = lt_pool.tile([112, UNIT], f32, tag="lt")
                for g in range(48):
                    gl = g if g < 30 else g - 30
                    p0 = 0 if g < 30 else 64
                    col = (gl // 10) * 512 + (gl % 10) * 48
                    if reorder:
                        mm_rhs = qts[b][:].rearrange("p (i j) -> p j i", j=48)[
                            :, g : g + 1, :
                        ]
                    else:
                        mm_rhs = qts[b][:, g * 48 : (g + 1) * 48]
                    nc.tensor.matmul(
                        pw[p0 : p0 + 48, col : col + 48],
                        relt[:, 47 - g : 95 - g],
                        mm_rhs,
                        start=True,
                        stop=True,
                    )
                dst = rhs[b][dst_rows[0] : dst_rows[1], :]
                for g0, p0, bank, cnt in (
                    (0, 0, 0, 10),
                    (10, 0, 1, 10),
                    (20, 0, 2, 10),
                    (30, 64, 0, 10),
                    (40, 64, 1, 8),
                ):
                    src = pw[p0 : p0 + 48, bank * 512 : bank * 512 + cnt * 48]
                    if reorder:
                        # psum cols are (j-slot, i); rhs wants n = i*48+j
                        nc.vector.tensor_copy(
                            dst.rearrange("p (i j) -> p i j", j=48)[
                                :, :, g0 : g0 + cnt
                            ],
                            src.rearrange("p (g i) -> p i g", i=48),
                        )
                    else:
                        nc.vector.tensor_copy(
                            dst[:, g0 * 48 : (g0 + cnt) * 48], src
                        )

            for b in range(B):
                asm(b, wrelt, (0, 48), True)
                asm(b, hrelt, (64, 112), False)

            # ---- main streamed loop ----
            for b in range(B):
                pv = pv_pool.tile([128, NB * VC], f32, tag="pv")
                for u in range(NU):
                    s0 = u * UNIT          # global stream start col
                    s1 = s0 + UNIT
                    lt = lt_pool.tile([128, UNIT], f32, tag="lt")
                    # pieces of this unit (may span two key-blocks)
                    c = s0
                    while c < s1:
                        j = c // HW
                        piece_end = min(s1, (j + 1) * HW)
                        # split further at psum 512-bank boundaries
                        cc = c
                        while cc < piece_end:
                            off = cc - s0
                            cut = min(piece_end, s0 + (off // 512 + 1) * 512)
                            nc.tensor.matmul(
                                lt[:, off : off + (cut - cc)],
                                lhs[b][:, 128 * j : 128 * (j + 1)],
                                rhs[b][:, cc - j * HW : cut - j * HW],
                                start=True,
                                stop=True,
                            )
                            cc = cut
                        c = piece_end
                    nc.scalar.activation(
                        ebuf[:, s0 % EBUF : s0 % EBUF + UNIT],
                        lt[:],
                        mybir.ActivationFunctionType.Exp,
                        bias=ebias[:],
                    )
                    # PV for every key-block fully exp'd by this unit
                    j = (s0 // HW) if s0 % HW else s0 // HW  # first block in unit
                    for jj in range(s0 // HW, (s1 - 1) // HW + 1):
                        if (jj + 1) * HW <= s1:
                            base = (jj * HW) % EBUF
                            for i in range(NB):
                                nc.tensor.matmul(
                                    pv[:, VC * i : VC * (i + 1)],
                                    ebuf[:, base + 128 * i : base + 128 * (i + 1)],
                                    vps[b][:, VC * jj : VC * (jj + 1)],
                                    start=(jj == 0 and i == 0),
                                    stop=(jj == NB - 1 and i == NB - 1),
                                )

                # ---- normalize + store ----
                rec = fin_pool.tile([128, NB], f32, tag="rec")
                nc.vector.reciprocal(rec[:], pv[:, 8 :: VC])
                outs = fin_pool.tile([128, NB * VC], f32, tag="outs")
                nc.vector.tensor_mul(
                    outs[:].rearrange("p (t d) -> p t d", d=VC),
                    pv[:].rearrange("p (t d) -> p t d", d=VC),
                    rec[:].unsqueeze(2).to_broadcast([128, NB, VC]),
                )
                nc.gpsimd.dma_start(
                    out_d.ap()[b].rearrange("(t p) d -> p t d", p=128),
                    outs[:].rearrange("p (t d) -> p t d", d=VC)[:, :, 0:8],
                )

    nc.compile()
    return nc


def _get_program():
    if "nc" not in _CACHE:
        _CACHE["nc"] = _build_program()
    return _CACHE["nc"]


def _prep_in_maps(inputs, key_rel_w, key_rel_h):
    """Host-side shard + layout prep. Returns list of 8 in_maps."""
    x = np.asarray(inputs, np.float32).reshape(B, HW, 192)
    scale = np.float32(DKH**-0.5)

    m = np.arange(HW)
    r = np.arange(48)[:, None]
    eyz = np.zeros((64, HW), np.float16)
    eyz[0:48] = m[None, :] % 48 == r
    ex = (m[None, :] // 48 == r).astype(np.float16)
    zz = np.zeros((16, HW), np.float16)
    wrelt = np.ascontiguousarray(np.asarray(key_rel_w, np.float32).T).astype(np.float16)
    hrelt = np.ascontiguousarray(np.asarray(key_rel_h, np.float32).T).astype(np.float16)

    in_maps = []
    for h in range(NH):
        q = x[:, :, 8 * h : 8 * h + 8] * scale
        k = x[:, :, 64 + 8 * h : 64 + 8 * h + 8]
        v = x[:, :, 128 + 8 * h : 128 + 8 * h + 8]
        qt = np.ascontiguousarray(q.transpose(0, 2, 1)).astype(np.float16)
        kt = np.ascontiguousarray(k.transpose(0, 2, 1)).astype(np.float16)
        vp = np.ones((B, NB, 128, VC), np.float16)
        vp[:, :, :, 0:8] = v.reshape(B, NB, 128, 8)
        vp = np.ascontiguousarray(vp.transpose(0, 2, 1, 3)).reshape(B, 128, NB * VC)
        in_maps.append(
            {
                "qt": qt,
                "kt": kt,
                "vp": vp,
                "eyz": eyz,
                "ex": ex,
                "zz": zz,
                "wrelt": wrelt,
                "hrelt": hrelt,
            }
        )
    return in_maps


def _run(in_maps, trace=False, tmpdir=None):
    from concourse import bass_utils

    nc = _get_program()
    return bass_utils.run_bass_kernel_spmd(
        nc,
        in_maps,
        core_ids=list(range(NH)),
        trace=trace,
        tmpdir=tmpdir,
    )


def kernel(inputs, key_rel_w, key_rel_h):
    in_maps = _prep_in_maps(inputs, key_rel_w, key_rel_h)
    res = _run(in_maps)
    out = np.empty((B, HW, 64), np.float32)
    for c in range(NH):
        out[:, :, 8 * c : 8 * (c + 1)] = res.results[c]["out"]
    return out.reshape(B, HH, WW, 64)


# revision 22
# speedup vs baseline: 1314.2374x; 1314.2374x over previous
"""Trainium2 Bass kernel for AttentionAugmentation2D.

Problem: 2D self-attention with relative position logits.
  inputs [2, 48, 48, 192] -> q,k,v (64 ch each), 8 heads x 8 dims.
  Per (batch, head): logits[n, m] = q.k + w_rel + h_rel over HW=2304 positions,
  softmax over m, output = weights @ v.

Sharding: 16 independent (batch, head) units -> head h goes to core h,
each core handles both batches for its head (2 units/core).

Math trick: for key m and query n (n = i*48+j, m = i'*48+j'):
  logitsT[m, n] = sum_d k[m,d] q[n,d]
               + sum_d key_rel_w[47 + j'(m) - j(n), d] q[n,d]
               + sum_d key_rel_h[47 + i'(m) - i(n), d] q[n,d]
One matmul with contraction K=120:
  lhsT rows: [EY (48) | 0 (16) | EX (48) | kT (8)]  EY[y,m]=[m%48==y], EX[x,m]=[m//48==x]
  rhs  rows: [RW (48) | 0 (16) | RH (48) | qT (8)]  RW[y,n] = sum_d Wrel[47+y-j(n),d] q[n,d]
(48-row blocks at 32-aligned partition bases for the DVE copies that fill
them; kT/qT rows are DMA-written so any partition base is fine there.)
RW/RH are built on-chip by 48 small matmuls each (per j-class / i-class of n),
stacked into one [112, 1536] psum tile (30 groups at partitions 0.., 18 at 64..).

The logits stream (18 key-blocks x 2304 queries) is produced in uniform
[128, 1536] psum units that roll across block boundaries; each unit gets one
exp ACT op into a rolling [128, 4608] fp16 buffer (block j's exps always land
contiguously at (2304*j) % 4608).  Softmax skips max-subtraction (|logits| <~
12; exp biased by -4 which cancels in normalization).  PV matmuls accumulate
[attn_unnorm | rowsum] per 128-query block via a ones-augmented v (fp16).
"""

import sys

if "/opt/trn_rl_repo" not in sys.path:
    sys.path.insert(0, "/opt/trn_rl_repo")

import numpy as np

B = 2
HH = 48
WW = 48
NH = 8
DKH = 8
HW = HH * WW          # 2304
NB = HW // 128        # 18 query/key blocks
KDIM = 120            # contraction rows: RW 0:48, zero 48:64, RH 64:112, qT 112:120
VC = 9                # v cols + ones
UNIT = 1536           # psum unit width (3 banks)
NU = NB * HW // UNIT  # 27 units per (batch, head) pair
EBUF = 2 * HW         # rolling exp buffer cols
EXP_BIAS = -4.0

_CACHE = {}


def _build_program():
    import concourse.bass as bass
    from concourse import bacc, mybir, tile

    f16 = mybir.dt.float16
    f32 = mybir.dt.float32

    nc = bacc.Bacc(
        "TRN2",
        target_bir_lowering=False,
        debug=False,
        enable_asserts=False,
        num_devices=8,
    )

    qt_d = nc.dram_tensor("qt", [B, 8, HW], f16, kind="ExternalInput")
    kt_d = nc.dram_tensor("kt", [B, 8, HW], f16, kind="ExternalInput")
    vp_d = nc.dram_tensor("vp", [B, 128, NB * VC], f16, kind="ExternalInput")
    eyz_d = nc.dram_tensor("eyz", [64, HW], f16, kind="ExternalInput")
    ex_d = nc.dram_tensor("ex", [48, HW], f16, kind="ExternalInput")
    zz_d = nc.dram_tensor("zz", [16, HW], f16, kind="ExternalInput")
    wrelt_d = nc.dram_tensor("wrelt", [8, 95], f16, kind="ExternalInput")
    hrelt_d = nc.dram_tensor("hrelt", [8, 95], f16, kind="ExternalInput")
    out_d = nc.dram_tensor("out", [B, HW, 8], f32, kind="ExternalOutput")

    with tile.TileContext(nc) as tc:
        with (
            tc.tile_pool(name="per", bufs=1) as per_pool,
            tc.tile_pool(name="fin", bufs=2) as fin_pool,
            tc.tile_pool(name="lt", bufs=2, space="PSUM") as lt_pool,
            tc.tile_pool(name="pv", bufs=2, space="PSUM") as pv_pool,
        ):
            wrelt = per_pool.tile([8, 95], f16, tag="wrelt")
            hrelt = per_pool.tile([8, 95], f16, tag="hrelt")
            ebias = per_pool.tile([128, 1], f32, tag="ebias")
            ebuf = per_pool.tile([128, EBUF], f16, tag="ebuf")
            lhs = [
                per_pool.tile([KDIM, HW], f16, tag=f"lhs{b}", name=f"lhs{b}")
                for b in range(B)
            ]
            rhs = [
                per_pool.tile([KDIM, HW], f16, tag=f"rhs{b}", name=f"rhs{b}")
                for b in range(B)
            ]
            qts = [
                per_pool.tile([8, HW], f16, tag=f"qts{b}", name=f"qts{b}")
                for b in range(B)
            ]
            vps = [
                per_pool.tile([128, NB * VC], f16, tag=f"vps{b}", name=f"vps{b}")
                for b in range(B)
            ]

            nc.vector.memset(ebias[:], EXP_BIAS)

            # All input DMAs up front, spread over four DGE queues.
            nc.scalar.dma_start(qts[0][:], qt_d.ap()[0])
            nc.scalar.dma_start(qts[1][:], qt_d.ap()[1])
            nc.sync.dma_start(wrelt[:], wrelt_d.ap())
            nc.sync.dma_start(hrelt[:], hrelt_d.ap())
            nc.sync.dma_start(rhs[0][48:64, :], zz_d.ap())
            nc.sync.dma_start(rhs[0][112:120, :], qt_d.ap()[0])
            nc.sync.dma_start(lhs[0][0:64, :], eyz_d.ap())
            nc.sync.dma_start(lhs[0][64:112, :], ex_d.ap())
            nc.sync.dma_start(lhs[0][112:120, :], kt_d.ap()[0])
            nc.gpsimd.dma_start(vps[0][:], vp_d.ap()[0])
            nc.gpsimd.dma_start(rhs[1][48:64, :], zz_d.ap())
            nc.gpsimd.dma_start(rhs[1][112:120, :], qt_d.ap()[1])
            nc.gpsimd.dma_start(lhs[1][0:64, :], eyz_d.ap())
            nc.gpsimd.dma_start(lhs[1][64:112, :], ex_d.ap())
            nc.gpsimd.dma_start(lhs[1][112:120, :], kt_d.ap()[1])
            nc.gpsimd.dma_start(vps[1][:], vp_d.ap()[1])

            # ---- RW/RH assembly for both pairs (overlaps main loop startup).
            # 48 groups per table stacked in one [112,1536] psum tile:
            # group g<30 -> partitions 0:48, bank g//10, slot g%10;
            # g>=30 -> partitions 64:112, bank (g-30)//10, slot (g-30)%10.
            def asm_small(b, relt, dst_rows, reorder, t):
                """One 20-group RW/RH assembly tile through the dedicated
                1-bank asm pool (10 groups at partitions 0.., 10 at 64..).
                Interleaved into the main unit stream spaced far enough apart
                that the slot is always free when the matmuls issue (PE runs
                its queue in order, so a slot-wait here would stall units)."""
                dst = rhs[b][dst_rows[0] : dst_rows[1], :]
                if True:
                    pw = asm_pool.tile(
                        [112, 512], f32, tag="asm", name=f"asm{b}_{dst_rows[0]}_{t}"
                    )
                    spans = [
                        (20 * t, min(20 * t + 10, 48), 0),
                        (20 * t + 10, min(20 * t + 20, 48), 64),
                    ]
                    for g0, g1, p0 in spans:
                        for g in range(g0, g1):
                            col = (g - g0) * 48
                            if reorder:
                                mm_rhs = qts[b][:].rearrange(
                                    "p (i j) -> p j i", j=48
                                )[:, g : g + 1, :]
                            else:
                                mm_rhs = qts[b][:, g * 48 : (g + 1) * 48]
                            nc.tensor.matmul(
                                pw[p0 : p0 + 48, col : col + 48],
                                relt[:, 47 - g : 95 - g],
                                mm_rhs,
                                start=True,
                                stop=True,
                            )
                    for g0, g1, p0 in spans:
                        cnt = g1 - g0
                        if cnt <= 0:
                            continue
                        src = pw[p0 : p0 + 48, 0 : cnt * 48]
                        if reorder:
                            nc.vector.tensor_copy(
                                dst.rearrange("p (i j) -> p i j", j=48)[
                                    :, :, g0:g1
                                ],
                                src.rearrange("p (g i) -> p i g", i=48),
                            )
                        else:
                            nc.vector.tensor_copy(
                                dst[:, g0 * 48 : g1 * 48], src
                            )

            def asm(b, relt, dst_rows, reorder, use_act):
                pw = lt_pool.tile([112, UNIT], f32, tag="lt")
                for g in range(48):
                    gl = g if g < 30 else g - 30
                    p0 = 0 if g < 30 else 64
                    col = (gl // 10) * 512 + (gl % 10) * 48
                    if reorder:
                        mm_rhs = qts[b][:].rearrange("p (i j) -> p j i", j=48)[
                            :, g : g + 1, :
                        ]
                    else:
                        mm_rhs = qts[b][:, g * 48 : (g + 1) * 48]
                    nc.tensor.matmul(
                        pw[p0 : p0 + 48, col : col + 48],
                        relt[:, 47 - g : 95 - g],
                        mm_rhs,
                        start=True,
                        stop=True,
                    )
                dst = rhs[b][dst_rows[0] : dst_rows[1], :]
                copy = nc.scalar.copy if use_act else nc.vector.tensor_copy
                # low partitions: groups 0..29 over 3 banks in one 4D copy.
                # src dims (bank:512-stride, g:48-stride, i/j:1-stride)
                src_low = pw[0:48, :].rearrange("p (bank r) -> p bank r", bank=3)[
                    :, :, 0:480
                ].rearrange("p bank (g i) -> p bank g i", g=10)
                if reorder:
                    # dst n = i*48 + (bank*10 + g)
                    dst_low = dst.rearrange("p (i j) -> p i j", j=48)[
                        :, :, 0:30
                    ].rearrange("p i (bank g) -> p bank g i", bank=3)
                else:
                    # dst n = (bank*10 + g)*48 + j, contiguous
                    dst_low = dst[:, 0:1440].rearrange(
                        "p (bank g j) -> p bank g j", bank=3, g=10
                    )
                copy(dst_low, src_low)
                for g0, bank, cnt in ((30, 0, 10), (40, 1, 8)):
                    src = pw[64:112, bank * 512 : bank * 512 + cnt * 48]
                    if reorder:
                        copy(
                            dst.rearrange("p (i j) -> p i j", j=48)[
                                :, :, g0 : g0 + cnt
                            ],
                            src.rearrange("p (g i) -> p i g", i=48),
                        )
                    else:
                        copy(dst[:, g0 * 48 : (g0 + cnt) * 48], src)

            def emit_unit(b, u, pv):
                s0 = u * UNIT          # global stream start col
                s1 = s0 + UNIT
                lt = lt_pool.tile([128, UNIT], f32, tag="lt", name=f"lt{b}_{u}")
                # pieces of this unit (may span two key-blocks)
                c = s0
                while c < s1:
                    j = c // HW
                    piece_end = min(s1, (j + 1) * HW)
                    # split further at psum 512-bank boundaries
                    cc = c
                    while cc < piece_end:
                        off = cc - s0
                        cut = min(piece_end, s0 + (off // 512 + 1) * 512)
                        nc.tensor.matmul(
                            lt[:, off : off + (cut - cc)],
                            lhs[b][:, 128 * j : 128 * (j + 1)],
                            rhs[b][:, cc - j * HW : cut - j * HW],
                            start=True,
                            stop=True,
                        )
                        cc = cut
                    c = piece_end
                nc.scalar.activation(
                    ebuf[:, s0 % EBUF : s0 % EBUF + UNIT],
                    lt[:],
                    mybir.ActivationFunctionType.Exp,
                    bias=ebias[:],
                )
                # PV for every key-block fully exp'd by this unit
                for jj in range(s0 // HW, (s1 - 1) // HW + 1):
                    if (jj + 1) * HW <= s1:
                        base = (jj * HW) % EBUF
                        for i in range(NB):
                            nc.tensor.matmul(
                                pv[:, VC * i : VC * (i + 1)],
                                ebuf[:, base + 128 * i : base + 128 * (i + 1)],
                                vps[b][:, VC * jj : VC * (jj + 1)],
                                start=(jj == 0 and i == 0),
                                stop=(jj == NB - 1 and i == NB - 1),
                            )

            def finish_pair(b, pv):
                rec = fin_pool.tile([128, NB], f32, tag="rec", name=f"rec{b}")
                nc.vector.reciprocal(rec[:], pv[:, 8 :: VC])
                outs = fin_pool.tile([128, NB * VC], f32, tag="outs", name=f"outs{b}")
                nc.vector.tensor_mul(
                    outs[:].rearrange("p (t d) -> p t d", d=VC),
                    pv[:].rearrange("p (t d) -> p t d", d=VC),
                    rec[:].unsqueeze(2).to_broadcast([128, NB, VC]),
                )
                nc.gpsimd.dma_start(
                    out_d.ap()[b].rearrange("(t p) d -> p t d", p=128),
                    outs[:].rearrange("p (t d) -> p t d", d=VC)[:, :, 0:8],
                )

            # pair 0 assembly first (its copies split DVE/ACT while ACT is
            # idle), then pair 0's stream with pair 1's assembly interleaved
            # after the pipeline is primed (copies all on DVE, which is idle
            # in steady state), then pair 1's stream.
            asm(0, wrelt, (0, 48), True, use_act=False)
            asm(0, hrelt, (64, 112), False, use_act=True)

            asm(1, wrelt, (0, 48), True, use_act=False)
            asm(1, hrelt, (64, 112), False, use_act=False)

            pv0 = pv_pool.tile([128, NB * VC], f32, tag="pv", name="pv0")
            for u in range(NU):
                emit_unit(0, u, pv0)
            finish_pair(0, pv0)

            pv1 = pv_pool.tile([128, NB * VC], f32, tag="pv", name="pv1")
            for u in range(NU):
                emit_unit(1, u, pv1)
            finish_pair(1, pv1)

    nc.compile()
    return nc


def _get_program():
    if "nc" not in _CACHE:
        _CACHE["nc"] = _build_program()
    return _CACHE["nc"]


def _prep_in_maps(inputs, key_rel_w, key_rel_h):
    """Host-side shard + layout prep. Returns list of 8 in_maps."""
    x = np.asarray(inputs, np.float32).reshape(B, HW, 192)
    scale = np.float32(DKH**-0.5)

    m = np.arange(HW)
    r = np.arange(48)[:, None]
    eyz = np.zeros((64, HW), np.float16)
    eyz[0:48] = m[None, :] % 48 == r
    ex = (m[None, :] // 48 == r).astype(np.float16)
    zz = np.zeros((16, HW), np.float16)
    wrelt = np.ascontiguousarray(np.asarray(key_rel_w, np.float32).T).astype(np.float16)
    hrelt = np.ascontiguousarray(np.asarray(key_rel_h, np.float32).T).astype(np.float16)

    in_maps = []
    for h in range(NH):
        q = x[:, :, 8 * h : 8 * h + 8] * scale
        k = x[:, :, 64 + 8 * h : 64 + 8 * h + 8]
        v = x[:, :, 128 + 8 * h : 128 + 8 * h + 8]
        qt = np.ascontiguousarray(q.transpose(0, 2, 1)).astype(np.float16)
        kt = np.ascontiguousarray(k.transpose(0, 2, 1)).astype(np.float16)
        vp = np.ones((B, NB, 128, VC), np.float16)
        vp[:, :, :, 0:8] = v.reshape(B, NB, 128, 8)
        vp = np.ascontiguousarray(vp.transpose(0, 2, 1, 3)).reshape(B, 128, NB * VC)
        in_maps.append(
            {
                "qt": qt,
                "kt": kt,
                "vp": vp,
                "eyz": eyz,
                "ex": ex,
                "zz": zz,
                "wrelt": wrelt,
                "hrelt": hrelt,
            }
        )
    return in_maps


def _run(in_maps, trace=False, tmpdir=None):
    from concourse import bass_utils

    nc = _get_program()
    return bass_utils.run_bass_kernel_spmd(
        nc,
        in_maps,
        core_ids=list(range(NH)),
        trace=trace,
        tmpdir=tmpdir,
    )


def kernel(inputs, key_rel_w, key_rel_h):
    in_maps = _prep_in_maps(inputs, key_rel_w, key_rel_h)
    res = _run(in_maps)
    out = np.empty((B, HW, 64), np.float32)
    for c in range(NH):
        out[:, :, 8 * c : 8 * (c + 1)] = res.results[c]["out"]
    return out.reshape(B, HH, WW, 64)
